# revision 22
# baseline (speedup 1.0000x reference)
"""Trainium2 Bass kernel for nn_AllModel_35828617183965 (prototypical networks).

Self-contained. Host folds BN into conv/fc weights and computes a shifted-
activation bias cascade; the device graph is a single-core Bass/Tile pipeline
(no collectives) run SPMD over 8 NeuronCores, 4 episodes per core.

v8: software-pipelined main loop. Each step emits conv1(s), conv2(s-1),
conv3(s-2), tree(s-3) so that PSUM drains of one layer overlap matmuls of
the others and every engine always has ready work one stage behind (v3 was
dependency-latency bound at ~26-47% engine occupancy; this layout measured
~557us vs 935us for the per-burst-serialized v3).

Layout (per core, 4 episodes = 1200 image slots, unchanged from v3):
  - episodes are interleaved across slots: image i of episode e sits at
    slot 4*i+e (supports in slots 0..799, queries 800..1199). 75 bursts
    of 16 images.
  - per burst: conv1 as 4x concurrent 32x128 PE tiles (block-diag
    weights), conv2 as 8x concurrent 32x64 tiles, conv3 as 2x concurrent
    64x128 tiles; 3 taps accumulate in PSUM; each layer uses two 2-bank
    PSUM tiles drained by one scalar activation (relu+bias, even columns)
    plus one vector scalar_tensor_tensor (max with odd columns) each.
  - SBUF guard columns (conv pad + pool pad) are written once per pool
    slot (first two steps) and persist across slot reuse.
  - avgpool: 3-level pairwise add tree on GPSIMD then a single FD=128
    vector tensor_reduce into `means`, 3 steps behind conv1.
  - means is stored in burst order m = 8*ti+4*h+2*s+b; an episode's
    columns are the fixed-stride set {16a + 4h + b + moff(e)}.
  - epilogue v8: all 4 episodes' protos are batched into 5 sub-stages at
    steps 52..56 (stage copies; 8 fc matmuls + bias via K=1 accumulating
    matmuls into ONE psum hold drained by one relu; square+reduce for the
    per-support norms; the inv-norm scales the one-hot rather than the
    features, so pm2 = -2*protos^T comes from matmuls on the unnormalized
    features into one more hold). The query tail is batched across all 4
    episodes (one fc, one relu, one square, one norm matmul, four small
    dp matmuls) and the final elementwise math (1/sqrt, distance
    assembly, sqrt, negation, de-permutation) runs on the host from the
    raw outputs u = qf^T pm2, nsq = |qf|^2, and pm2. A dummy Sqrt at t=0
    front-loads the ACT table set; const DMAs are in first-use order.
"""
import sys
import types
import numpy as np

sys.path.insert(0, '/opt/trn_rl_repo')

if 'antenv.axon_hooks' not in sys.modules:
    _m = types.ModuleType('antenv.axon_hooks')
    _m._hook = None
    def _set_hook(h, _m=_m):
        _m._hook = h
    def _get_hook(_m=_m):
        return _m._hook
    _m.set_axon_ntff_profile_hook = _set_hook
    _m.get_axon_ntff_profile_hook = _get_hook
    sys.modules['antenv.axon_hooks'] = _m

import ml_dtypes

F16 = np.float16
EPS_BN = 1e-5
N_WAY = 20
B, NS, NQ, C_IN, L0 = 32, 200, 100, 8, 512
NCORES = 8
B_LOC = B // NCORES          # 4 episodes per core
N_SUP = B_LOC * NS           # 800
N_QRY = B_LOC * NQ           # 400
NSLOT = N_SUP + N_QRY        # 1200
NBURST = NSLOT // 16         # 75


# ---------------- host math ----------------

def _fold_bn(w, b, g, be, m, v):
    scale = g / np.sqrt(v + EPS_BN)
    return w * scale[:, None, None], (b - m) * scale + be


def _prepare(inp):
    f = lambda k: np.asarray(inp[k], dtype=np.float64)
    W1, b1 = _fold_bn(f('w1'), f('b1'), f('g1'), f('be1'), f('m1'), f('v1'))
    W2, b2 = _fold_bn(f('w2'), f('b2'), f('g2'), f('be2'), f('m2'), f('v2'))
    W3, b3 = _fold_bn(f('w3'), f('b3'), f('g3'), f('be3'), f('m3'), f('v3'))
    sf = f('gf') / np.sqrt(f('vf') + EPS_BN)
    wf2 = (f('wf') * sf[:, None]).T              # [K=128, F=128]
    bf2 = (f('bf') - f('mf')) * sf + f('bef')
    c1 = b1
    c2 = b2 + W2.sum(axis=2) @ c1
    c3 = b3 + W3.sum(axis=2) @ c2
    bf3 = bf2 + wf2.T @ c3
    return dict(W1=W1, W2=W2, W3=W3, wf2=wf2, c1=c1, c2=c2, c3=c3, bf3=bf3)


def _layouts(p):
    w1 = np.zeros((128, 3 * 128), np.float64)
    for r in range(4):
        for t in range(3):
            for a in range(4):
                w1[32 * r + 8 * a: 32 * r + 8 * a + 8,
                   128 * t + 32 * a: 128 * t + 32 * a + 32] = p['W1'][:, :, t].T
    w2 = np.zeros((128, 3 * 64), np.float64)
    for s in range(4):
        for t in range(3):
            w2[32 * s: 32 * s + 32, 64 * t: 64 * t + 64] = p['W2'][:, :, t].T
    w3 = np.zeros((128, 3 * 128), np.float64)
    for h in range(2):
        for t in range(3):
            w3[64 * h: 64 * h + 64, 128 * t: 128 * t + 128] = p['W3'][:, :, t].T
    out = dict(w1=w1, w2=w2, w3=w3, wf=p['wf2'] / 64.0)
    out['negb1'] = np.tile(-p['c1'], 4)[:, None]
    out['negc2'] = np.tile(-p['c2'], 2)[:, None]
    out['negc3'] = (-p['c3'])[:, None]
    out['negc2_5'] = np.tile(out['negc2'], (1, 5))
    out['posb1'] = -out['negb1']
    out['posc2'] = -out['negc2']
    out['posc3'] = -out['negc3']
    out['negb1_4'] = np.tile(out['negb1'], (1, 4))
    out['bf3_row'] = p['bf3'][None, :]
    out['bf3_col'] = p['bf3'][:, None]
    return out


def _build_onehot_core(s_lab_core):
    """oh dram layout [128, B_LOC*2*20], scaled by -2 so the prototype
    matmul (lhsT=sf, rhs=inv*oh) directly yields pm2 = -2*protos^T.

    Row r of block (e, c) corresponds to the fc-lhsT stream order
    (a, h, b): support index s = 4*(a + 32*c) + 2*b + h."""
    oh = np.zeros((128, B_LOC * 2 * N_WAY), np.float64)
    for e in range(B_LOC):
        lab = np.asarray(s_lab_core[e])
        onehot = (lab[:, None] == np.arange(N_WAY)[None, :]).astype(np.float64)
        counts = onehot.sum(axis=0)
        counts[counts == 0] = 1.0
        ohn = -2.0 * onehot / counts[None, :]    # [200, 20]
        for c, (a0, na) in enumerate([(0, 32), (32, 18)]):
            blk = np.zeros((128, N_WAY))
            for r in range(4 * na):
                a, hb = a0 + r // 4, r % 4
                h, b = hb // 2, hb % 2
                blk[r] = ohn[4 * a + 2 * b + h]
            oh[:, (2 * e + c) * N_WAY:(2 * e + c + 1) * N_WAY] = blk
    return oh


# ---------------- device graph ----------------

_CACHE = {}


def _build_nc():
    import concourse.bass as bass
    import concourse.tile as tile
    from concourse import bacc, mybir
    from contextlib import ExitStack
    dt = mybir.dt
    AO = mybir.AluOpType
    AF = mybir.ActivationFunctionType

    nc = bacc.Bacc("TRN2", target_bir_lowering=False, debug=False,
                   num_devices=NCORES)

    dram = {}
    def din(name, shape, dtype):
        dram[name] = nc.dram_tensor(name, list(shape), dtype, kind="ExternalInput")

    din('imgs', (NSLOT, C_IN, L0 + 2), dt.float16)
    din('w1', (128, 384), dt.float16)
    din('w2', (128, 192), dt.float16)
    din('w3', (128, 384), dt.float16)
    din('wf', (128, 128), dt.float32)
    din('negb1', (128, 1), dt.float32)
    din('posb1', (128, 1), dt.float32)
    din('posc2', (128, 1), dt.float32)
    din('posc3', (128, 1), dt.float32)
    din('negb1_4', (128, 4), dt.float16)
    din('negc2', (128, 1), dt.float32)
    din('negc3', (128, 1), dt.float32)
    din('negc2_5', (128, 5), dt.float16)
    din('bf3_row', (1, 128), dt.float32)
    din('bf3_col', (128, 1), dt.float32)
    din('oh', (128, B_LOC * 2 * N_WAY), dt.float32)
    din('onesrow_f', (1, 128), dt.float32)
    din('ones128_f', (128, 1), dt.float32)
    u_d = nc.dram_tensor('u', [NQ, B_LOC * N_WAY], dt.float32,
                         kind="ExternalOutput")
    nsq_d = nc.dram_tensor('nsq', [1, N_QRY], dt.float32,
                           kind="ExternalOutput")
    pm2_d = nc.dram_tensor('pm2', [128, B_LOC * N_WAY], dt.float32,
                           kind="ExternalOutput")

    with tile.TileContext(nc) as tc, ExitStack() as ctx:
        cpool = ctx.enter_context(tc.tile_pool(name="consts", bufs=1))
        cs = {}
        # DMA order = first-use order so the pipeline starts ASAP
        for name, shape, d in [
            ('w1', (128, 384), dt.float16), ('negb1_4', (128, 4), dt.float16),
            ('posb1', (128, 1), dt.float32), ('negb1', (128, 1), dt.float32),
            ('w2', (128, 192), dt.float16), ('negc2_5', (128, 5), dt.float16),
            ('posc2', (128, 1), dt.float32), ('negc2', (128, 1), dt.float32),
            ('w3', (128, 384), dt.float16),
            ('posc3', (128, 1), dt.float32), ('negc3', (128, 1), dt.float32),
            ('wf', (128, 128), dt.float32),
            ('bf3_row', (1, 128), dt.float32), ('bf3_col', (128, 1), dt.float32),
            ('oh', (128, B_LOC * 2 * N_WAY), dt.float32),
            ('onesrow_f', (1, 128), dt.float32), ('ones128_f', (128, 1), dt.float32),
        ]:
            t = cpool.tile(list(shape), d, tag=f"c_{name}")
            nc.sync.dma_start(out=t[:], in_=dram[name].ap())
            cs[name] = t
        # dummy sqrt at t=0: front-load the ACT table set that holds both
        # Sqrt (protos) and the Relu/Square fillers, avoiding a ~2.7us
        # mid-kernel table switch
        warm = cpool.tile([1, 1], dt.float32, tag="warm")
        nc.scalar.activation(out=warm[0:1, 0:1], in_=cs['onesrow_f'][0:1, 0:1],
                             func=AF.Sqrt)

        means = cpool.tile([128, NSLOT], dt.float32, tag="means")
        # means column view by (a, ti, h, s, b): col = 16a+8ti+4h+2s+b
        mview = means[:].rearrange("p (a ti h s b) -> p a ti h s b",
                                   ti=2, h=2, s=2, b=2)

        img_pool = ctx.enter_context(tc.tile_pool(name="img", bufs=3))
        p1_pool = ctx.enter_context(tc.tile_pool(name="p1", bufs=2))
        l3r_pool = ctx.enter_context(tc.tile_pool(name="l3r", bufs=2))
        p3_pool = ctx.enter_context(tc.tile_pool(name="p3", bufs=2))
        str_pool = ctx.enter_context(tc.tile_pool(name="str", bufs=2))
        tmp_pool = ctx.enter_context(tc.tile_pool(name="tmp", bufs=4))
        ps_pool = ctx.enter_context(tc.tile_pool(name="ps", bufs=4, space="PSUM"))
        ep_pool = ctx.enter_context(tc.tile_pool(name="ep", bufs=2))
        sf_pool = ctx.enter_context(tc.tile_pool(name="sf", bufs=2))
        pm2_pool = ctx.enter_context(tc.tile_pool(name="pm2", bufs=4))
        q_pool = ctx.enter_context(tc.tile_pool(name="q", bufs=1))

        imgs, p1s, Ts, p3s = {}, {}, {}, {}

        def emit_load(b):
            img = img_pool.tile([128, L0 + 2], dt.float16, tag="img")
            nc.sync.dma_start(out=img[:, :],
                              in_=dram['imgs'].ap()[b * 16:b * 16 + 16]
                              .rearrange("i c l -> (i c) l"))
            imgs[b] = img

        def emit_conv1(b):
            img = imgs.pop(b)
            p1 = p1_pool.tile([128, 4 * 258], dt.float16, tag="pooled1")
            p1v = p1[:].rearrange("p (r c) -> p r c", r=4)
            if b < 2:
                # guard columns: written once per slot, persist across reuse
                nc.gpsimd.tensor_copy(out=p1v[:, :, 0], in_=cs['negb1_4'][:, 0:4])
                nc.gpsimd.tensor_copy(out=p1v[:, :, 257], in_=cs['negb1_4'][:, 0:4])
            for rp in range(2):
                ps1 = ps_pool.tile([128, 1024], dt.float32, tag="ps")
                for j in range(2):
                    r = 2 * rp + j
                    for d in range(3):
                        nc.tensor.matmul(
                            out=ps1[:, 512 * j:512 * j + 512],
                            lhsT=cs['w1'][32 * r:32 * r + 32,
                                          128 * d:128 * d + 128],
                            rhs=img[32 * r:32 * r + 32, d:d + 512],
                            start=(d == 0), stop=(d == 2),
                            tile_position=(32 * r, 0))
                v1 = ps1.rearrange("p (j l) -> p j l", j=2)
                t1 = tmp_pool.tile([128, 512], dt.float16, tag="tmp")
                t1v = t1[:].rearrange("p (j l) -> p j l", j=2)
                nc.scalar.activation(out=t1v[:, :, :], in_=v1[:, :, 0:512:2],
                                     func=AF.Relu, bias=cs['posb1'][:, 0:1])
                nc.vector.scalar_tensor_tensor(
                    out=p1v[:, 2 * rp:2 * rp + 2, 1:257], in0=t1v[:, :, :],
                    scalar=cs['negb1'][:, 0:1], in1=v1[:, :, 1:512:2],
                    op0=AO.add, op1=AO.max)
            p1s[b] = p1

        def emit_conv2(b):
            p1 = p1s.pop(b)
            p1v = p1[:].rearrange("p (r c) -> p r c", r=4)
            T = l3r_pool.tile([128, 1036], dt.float16, tag="l3rhs")
            if b < 2:
                for ti in range(2):
                    nc.gpsimd.tensor_copy(out=T[:, 518 * ti:518 * ti + 517:129],
                                          in_=cs['negc2_5'][:, 0:5])
            for t2i in range(2):
                ps2 = ps_pool.tile([128, 1024], dt.float32, tag="ps")
                for sj in range(2):
                    s = 2 * t2i + sj
                    p1sv = p1v[32 * s:32 * s + 32]
                    for c in range(2):
                        for d in range(3):
                            nc.tensor.matmul(
                                out=ps2[64 * c:64 * c + 64,
                                        512 * sj:512 * sj + 512],
                                lhsT=cs['w2'][32 * s:32 * s + 32,
                                              64 * d:64 * d + 64],
                                rhs=p1sv[:, c:4:2, d:d + 256],
                                start=(d == 0), stop=(d == 2),
                                tile_position=(32 * s, 64 * c))
                src2 = ps2.rearrange("p (x b l) -> p x b l", x=2, b=2)
                t2 = tmp_pool.tile([128, 512], dt.float16, tag="tmp")
                t2v = t2[:].rearrange("p (x b l) -> p x b l", x=2, b=2)
                nc.scalar.activation(out=t2v[:, :, :, :],
                                     in_=src2[:, :, :, 0:256:2],
                                     func=AF.Relu, bias=cs['posc2'][:, 0:1])
                dst = T[:, 518 * t2i + 1:518 * t2i + 517].rearrange(
                    "p (x b l) -> p x b l", x=2, b=2)
                nc.vector.scalar_tensor_tensor(
                    out=dst[:, :, :, 0:128], in0=t2v[:, :, :, :],
                    scalar=cs['negc2'][:, 0:1], in1=src2[:, :, :, 1:256:2],
                    op0=AO.add, op1=AO.max)
            Ts[b] = T

        def emit_conv3(b):
            T = Ts.pop(b)
            p3 = p3_pool.tile([128, 1024], dt.float32, tag="p3")
            for ti in range(2):
                ps3 = ps_pool.tile([128, 1024], dt.float32, tag="ps")
                for h in range(2):
                    rhs3 = T[64 * h:64 * h + 64, 518 * ti:518 * ti + 518]
                    for d in range(3):
                        rv = rhs3[:, d:d + 516].rearrange(
                            "p (k l) -> p k l", k=4)[:, :, 0:128]
                        nc.tensor.matmul(
                            out=ps3[:, 512 * h:512 * h + 512],
                            lhsT=cs['w3'][64 * h:64 * h + 64,
                                          128 * d:128 * d + 128],
                            rhs=rv,
                            start=(d == 0), stop=(d == 2),
                            tile_position=(64 * h, 0))
                s3v = ps3.rearrange("p (h k l) -> p h k l", h=2, k=4)
                t3 = tmp_pool.tile([128, 512], dt.float16, tag="tmp")
                t3v = t3[:].rearrange("p (h k l) -> p h k l", h=2, k=4)
                nc.scalar.activation(out=t3v[:, :, :, :],
                                     in_=s3v[:, :, :, 0:128:2],
                                     func=AF.Relu, bias=cs['posc3'][:, 0:1])
                d3 = p3[:, 512 * ti:512 * ti + 512].rearrange(
                    "p (h k l) -> p h k l", h=2, k=4)
                nc.vector.scalar_tensor_tensor(
                    out=d3[:, :, :, 0:64], in0=t3v[:, :, :, :],
                    scalar=cs['negc3'][:, 0:1], in1=s3v[:, :, :, 1:128:2],
                    op0=AO.add, op1=AO.max)
            p3s[b] = p3

        def emit_tree(bb):
            """avgpool add-tree for burst bb (p3 -> means cols m-order)."""
            p3 = p3s.pop(bb)
            p3v = p3.rearrange("p (m l) -> p m l", m=16)
            s = str_pool.tile([128, 896], dt.float32, tag="scratch")
            s1 = s[:, 0:512].rearrange("p (m l) -> p m l", m=16)
            s2 = s[:, 512:768].rearrange("p (m l) -> p m l", m=16)
            s3 = s[:, 768:896].rearrange("p (m l) -> p m l", m=16)
            nc.gpsimd.tensor_tensor(out=s1[:, :, :], in0=p3v[:, :, 0:32],
                                    in1=p3v[:, :, 32:64], op=AO.add)
            nc.gpsimd.tensor_tensor(out=s2[:, :, :], in0=s1[:, :, 0:16],
                                    in1=s1[:, :, 16:32], op=AO.add)
            nc.gpsimd.tensor_tensor(out=s3[:, :, :], in0=s2[:, :, 0:8],
                                    in1=s2[:, :, 8:16], op=AO.add)
            nc.vector.tensor_reduce(out=means[:, 16 * bb:16 * bb + 16],
                                    in_=s3[:, :, :],
                                    axis=mybir.AxisListType.X, op=AO.add)

        # ---------------- epilogue: protos in 4 spread sub-stages ----------

        pr = {}

        def emit_proto_stage(st):
            if st == 0:
                # stage all 4 episodes' support means contiguously:
                # col (a,h,b) -> support index 4a+2b+h
                stage = ep_pool.tile([128, 4 * 200], dt.float32, tag="stage")
                for e in range(B_LOC):
                    tie, se = (e >> 1) & 1, e & 1
                    nc.gpsimd.tensor_copy(
                        out=stage[:, 200 * e:200 * e + 200].rearrange(
                            "p (a h b) -> p a h b", h=2, b=2),
                        in_=mview[:, 0:50, tie, :, se, :])
                pr['stage'] = stage
            elif st == 1:
                # 8 fc matmuls (block = 2e+c) + bias via K=1 accumulating
                # matmul, all into one 2-bank hold; one relu drains it
                hold = ps_pool.tile([128, 1024], dt.float32, tag="ps")
                for e in range(B_LOC):
                    for c, (off, sz) in enumerate([(0, 128), (128, 72)]):
                        blk = 2 * e + c
                        nc.tensor.matmul(
                            out=hold[0:sz, 128 * blk:128 * blk + 128],
                            lhsT=pr['stage'][:, 200 * e + off:200 * e + off + sz],
                            rhs=cs['wf'][:, 0:128], start=True, stop=False)
                        nc.tensor.matmul(
                            out=hold[0:sz, 128 * blk:128 * blk + 128],
                            lhsT=cs['onesrow_f'][0:1, 0:sz],
                            rhs=cs['bf3_row'][0:1, 0:128],
                            start=False, stop=True)
                sf = sf_pool.tile([128, 1024], dt.float32, tag="sf")
                nc.scalar.activation(out=sf[:, :], in_=hold[0:128, 0:1024],
                                     func=AF.Relu)
                pr['sf'] = sf
            elif st == 2:
                sq = ep_pool.tile([128, 1024], dt.float32, tag="sq")
                nc.gpsimd.tensor_tensor(out=sq[:, :], in0=pr['sf'][:, :],
                                        in1=pr['sf'][:, :], op=AO.mult)
                nsq = ep_pool.tile([128, 8], dt.float32, tag="nsq")
                nc.vector.tensor_reduce(
                    out=nsq[:, :], in_=sq[:].rearrange("p (k f) -> p k f", k=8),
                    axis=mybir.AxisListType.X, op=AO.add)
                nrm = ep_pool.tile([128, 8], dt.float32, tag="nrm")
                nc.scalar.activation(out=nrm[:, :], in_=nsq[:, :], func=AF.Sqrt)
                inv = ep_pool.tile([128, 8], dt.float32, tag="inv")
                nc.vector.reciprocal(out=inv[:, :], in_=nrm[:, :])
                pr['inv'] = inv
            elif st == 3:
                # scale the one-hot by the per-support inv-norm instead of
                # normalizing the features: pm2 = sf^T @ (inv * oh)
                ohs = ep_pool.tile([128, 8 * N_WAY], dt.float32, tag="ohs")
                for blk in range(8):
                    sz = 128 if blk % 2 == 0 else 72
                    nc.vector.tensor_scalar(
                        out=ohs[0:sz, N_WAY * blk:N_WAY * blk + N_WAY],
                        in0=cs['oh'][0:sz, N_WAY * blk:N_WAY * blk + N_WAY],
                        scalar1=pr['inv'][0:sz, blk:blk + 1], scalar2=None,
                        op0=AO.mult)
                pr['ohs'] = ohs
            elif st == 4:
                hold = ps_pool.tile([128, 1024], dt.float32, tag="ps")
                for e in range(B_LOC):
                    for c, sz in [(0, 128), (1, 72)]:
                        blk = 2 * e + c
                        nc.tensor.matmul(
                            out=hold[0:128, N_WAY * e:N_WAY * e + N_WAY],
                            lhsT=pr['sf'][0:sz, 128 * blk:128 * blk + 128],
                            rhs=pr['ohs'][0:sz,
                                          N_WAY * blk:N_WAY * blk + N_WAY],
                            start=(c == 0), stop=(c == 1))
                pm2 = pm2_pool.tile([128, B_LOC * N_WAY], dt.float32, tag="pm2")
                nc.vector.tensor_copy(out=pm2[:, :],
                                      in_=hold[0:128, 0:B_LOC * N_WAY])
                nc.sync.dma_start(out=pm2_d.ap(), in_=pm2[:, :])
                pr['pm2'] = pm2

        def emit_query_tail():
            """Batched query tail: one fc/relu/sq/norm for all 4 episodes,
            then 4 tiny dp matmuls. Final elementwise math is host-side."""
            qstage = q_pool.tile([128, N_QRY], dt.float32, tag="qstage")
            for ti in range(2):
                ov = qstage[:, 200 * ti:200 * ti + 200].rearrange(
                    "p (s a h b) -> p a h s b", a=25, h=2, b=2)
                nc.vector.tensor_copy(out=ov, in_=mview[:, 50:75, ti, :, :, :])
            hold = ps_pool.tile([128, 1024], dt.float32, tag="ps")
            nc.tensor.matmul(out=hold[0:128, 0:N_QRY],
                             lhsT=cs['wf'][:, 0:128],
                             rhs=qstage[:, 0:N_QRY], start=True, stop=True)
            qf = q_pool.tile([128, N_QRY], dt.float32, tag="qf")
            nc.scalar.activation(out=qf[:, :], in_=hold[0:128, 0:N_QRY],
                                 func=AF.Relu, bias=cs['bf3_col'][:, 0:1])
            qsq = q_pool.tile([128, N_QRY], dt.float32, tag="qsq")
            nc.vector.tensor_tensor(out=qsq[:, :], in0=qf[:, :],
                                    in1=qf[:, :], op=AO.mult)
            hold2 = ps_pool.tile([128, 1024], dt.float32, tag="ps")
            nc.tensor.matmul(out=hold2[0:1, 0:N_QRY],
                             lhsT=cs['ones128_f'][:, 0:1],
                             rhs=qsq[:, :], start=True, stop=True)
            nsq_sb = q_pool.tile([1, N_QRY], dt.float32, tag="nsq_sb")
            nc.vector.tensor_copy(out=nsq_sb[:, :], in_=hold2[0:1, 0:N_QRY])
            nc.sync.dma_start(out=nsq_d.ap(), in_=nsq_sb[:, :])
            hold3 = ps_pool.tile([128, 1024], dt.float32, tag="ps")
            for e in range(B_LOC):
                nc.tensor.matmul(out=hold3[0:NQ, N_WAY * e:N_WAY * e + N_WAY],
                                 lhsT=qf[:, NQ * e:NQ * e + NQ],
                                 rhs=pr['pm2'][:, N_WAY * e:N_WAY * e + N_WAY],
                                 start=True, stop=True)
            u_sb = q_pool.tile([128, B_LOC * N_WAY], dt.float32, tag="u_sb")
            nc.vector.tensor_copy(out=u_sb[0:NQ, :],
                                  in_=hold3[0:NQ, 0:B_LOC * N_WAY])
            nc.sync.dma_start(out=u_d.ap(), in_=u_sb[0:NQ, :])

        # ---------------- software-pipelined main loop ----------------
        proto_at = {52: [0], 53: [1], 54: [2], 55: [3], 56: [4]}

        emit_load(0)
        emit_load(1)
        for s in range(NBURST + 3):
            if s + 2 < NBURST:
                emit_load(s + 2)
            if s < NBURST:
                emit_conv1(s)
            if 0 <= s - 1 < NBURST:
                emit_conv2(s - 1)
            if 0 <= s - 2 < NBURST:
                emit_conv3(s - 2)
            if 0 <= s - 3 < NBURST:
                emit_tree(s - 3)
            for st in proto_at.get(s, []):
                emit_proto_stage(st)
        emit_query_tail()

    nc.compile()
    return nc


def _host_inputs(inputs):
    p = _prepare(inputs)
    lay = _layouts(p)
    f32 = lambda a: np.ascontiguousarray(a, dtype=np.float32)
    b16 = lambda a: np.ascontiguousarray(np.asarray(a, np.float32).astype(F16))
    s_img = np.asarray(inputs['s_img'], np.float32)
    q_img = np.asarray(inputs['q_img'], np.float32)
    s_lab = np.asarray(inputs['s_lab'])
    common = {
        'w1': b16(lay['w1']), 'w2': b16(lay['w2']), 'w3': b16(lay['w3']),
        'wf': f32(lay['wf']),
        'negb1': f32(lay['negb1']), 'negc2': f32(lay['negc2']),
        'posb1': f32(lay['posb1']), 'posc2': f32(lay['posc2']),
        'posc3': f32(lay['posc3']), 'negb1_4': b16(lay['negb1_4']),
        'negc3': f32(lay['negc3']), 'negc2_5': b16(lay['negc2_5']),
        'bf3_row': f32(lay['bf3_row']), 'bf3_col': f32(lay['bf3_col']),
        'onesrow_f': f32(np.ones((1, 128))), 'ones128_f': f32(np.ones((128, 1))),
    }
    in_maps = []
    for i in range(NCORES):
        e0 = i * B_LOC
        m = dict(common)
        # slot 4*i+e <- image i of episode e (episode-interleaved)
        sup = s_img[e0:e0 + B_LOC].transpose(1, 0, 2, 3).reshape(N_SUP, C_IN, L0)
        qry = q_img[e0:e0 + B_LOC].transpose(1, 0, 2, 3).reshape(N_QRY, C_IN, L0)
        allimg = np.concatenate([sup, qry], axis=0)
        padded = np.zeros((NSLOT, C_IN, L0 + 2), np.float32)
        padded[:, :, 1:L0 + 1] = allimg
        m['imgs'] = b16(padded)
        m['oh'] = f32(_build_onehot_core(s_lab[e0:e0 + B_LOC]))
        in_maps.append(m)
    return in_maps


# query col c = 4a+2h+b for query q = 4a+2b+h (swap h<->b in the low bits)
_QCOL = np.array([4 * (q // 4) + 2 * (q & 1) + ((q >> 1) & 1)
                  for q in range(NQ)], dtype=np.int64)


def _host_finish(res_core):
    """Assemble -dist [B_LOC, NQ, N_WAY] from raw device outputs."""
    u = np.asarray(res_core['u'], np.float64)        # [NQ, 4*20], rows=col c
    nsq = np.asarray(res_core['nsq'], np.float64)[0]  # [400], (e-block, col c)
    pm2 = np.asarray(res_core['pm2'], np.float64)     # [128, 4*20]
    out = np.empty((B_LOC, NQ, N_WAY), np.float32)
    for e in range(B_LOC):
        rows = 0.25 * (pm2[:, 20 * e:20 * e + 20] ** 2).sum(axis=0) + 1.0
        inv = 1.0 / np.sqrt(np.maximum(nsq[100 * e + _QCOL], 1e-24))
        d2 = u[_QCOL, 20 * e:20 * e + 20] * inv[:, None] + rows[None, :]
        out[e] = -np.sqrt(np.maximum(d2, 0.0))
    return out


def _ensure_ntff_hook():
    try:
        from antenv.axon_hooks import (get_axon_ntff_profile_hook,
                                       set_axon_ntff_profile_hook)
        if get_axon_ntff_profile_hook() is None:
            from trn_agent_boot.trn_boot import _ntff_profile_via_ctypes
            set_axon_ntff_profile_hook(
                _ntff_profile_via_ctypes('/opt/axon/libaxon_pjrt.so'))
    except Exception as e:
        print('ntff hook setup failed:', e)


def _run(inputs, trace=False):
    from concourse.bass_utils import run_bass_kernel_spmd
    if trace:
        _ensure_ntff_hook()
    if 'nc' not in _CACHE:
        _CACHE['nc'] = _build_nc()
    nc = _CACHE['nc']
    in_maps = _host_inputs(inputs)
    res = run_bass_kernel_spmd(nc, in_maps, core_ids=list(range(NCORES)),
                               trace=trace)
    outs = [_host_finish(res.results[i]) for i in range(NCORES)]
    full = np.concatenate(outs, axis=0).astype(np.float32)
    return full, res


def kernel(**inputs):
    out, _ = _run(inputs, trace=False)
    return out


def run_traced(**inputs):
    return _run(inputs, trace=True)


# revision 23
# speedup vs baseline: 1.0024x; 1.0024x over previous
"""Trainium2 Bass kernel for nn_AllModel_35828617183965 (prototypical networks).

Self-contained. Host folds BN into conv/fc weights and computes a shifted-
activation bias cascade; the device graph is a single-core Bass/Tile pipeline
(no collectives) run SPMD over 8 NeuronCores, 4 episodes per core.

v8: software-pipelined main loop. Each step emits conv1(s), conv2(s-1),
conv3(s-2), tree(s-3) so that PSUM drains of one layer overlap matmuls of
the others and every engine always has ready work one stage behind (v3 was
dependency-latency bound at ~26-47% engine occupancy; this layout measured
~557us vs 935us for the per-burst-serialized v3).

Layout (per core, 4 episodes = 1200 image slots, unchanged from v3):
  - episodes are interleaved across slots: image i of episode e sits at
    slot 4*i+e (supports in slots 0..799, queries 800..1199). 75 bursts
    of 16 images.
  - per burst: conv1 as 4x concurrent 32x128 PE tiles (block-diag
    weights), conv2 as 8x concurrent 32x64 tiles, conv3 as 2x concurrent
    64x128 tiles; 3 taps accumulate in PSUM; each layer uses two 2-bank
    PSUM tiles drained by one scalar activation (relu+bias, even columns)
    plus one vector scalar_tensor_tensor (max with odd columns) each.
  - SBUF guard columns (conv pad + pool pad) are written once per pool
    slot (first two steps) and persist across slot reuse.
  - avgpool: 3-level pairwise add tree on GPSIMD then a single FD=128
    vector tensor_reduce into `means`, 3 steps behind conv1.
  - means is stored in burst order m = 8*ti+4*h+2*s+b; an episode's
    columns are the fixed-stride set {16a + 4h + b + moff(e)}.
  - epilogue v8: all 4 episodes' protos are batched into 5 sub-stages at
    steps 52..56 (stage copies; 8 fc matmuls + bias via K=1 accumulating
    matmuls into ONE psum hold drained by one relu; square+reduce for the
    per-support norms; the inv-norm scales the one-hot rather than the
    features, so pm2 = -2*protos^T comes from matmuls on the unnormalized
    features into one more hold). The query tail is batched across all 4
    episodes (one fc, one relu, one square, one norm matmul, four small
    dp matmuls) and the final elementwise math (1/sqrt, distance
    assembly, sqrt, negation, de-permutation) runs on the host from the
    raw outputs u = qf^T pm2, nsq = |qf|^2, and pm2. A dummy Sqrt at t=0
    front-loads the ACT table set; const DMAs are in first-use order.
"""
import sys
import types
import numpy as np

sys.path.insert(0, '/opt/trn_rl_repo')

if 'antenv.axon_hooks' not in sys.modules:
    _m = types.ModuleType('antenv.axon_hooks')
    _m._hook = None
    def _set_hook(h, _m=_m):
        _m._hook = h
    def _get_hook(_m=_m):
        return _m._hook
    _m.set_axon_ntff_profile_hook = _set_hook
    _m.get_axon_ntff_profile_hook = _get_hook
    sys.modules['antenv.axon_hooks'] = _m

import ml_dtypes

F16 = np.float16
EPS_BN = 1e-5
N_WAY = 20
B, NS, NQ, C_IN, L0 = 32, 200, 100, 8, 512
NCORES = 8
B_LOC = B // NCORES          # 4 episodes per core
N_SUP = B_LOC * NS           # 800
N_QRY = B_LOC * NQ           # 400
NSLOT = N_SUP + N_QRY        # 1200
NBURST = NSLOT // 16         # 75


# ---------------- host math ----------------

def _fold_bn(w, b, g, be, m, v):
    scale = g / np.sqrt(v + EPS_BN)
    return w * scale[:, None, None], (b - m) * scale + be


def _prepare(inp):
    f = lambda k: np.asarray(inp[k], dtype=np.float64)
    W1, b1 = _fold_bn(f('w1'), f('b1'), f('g1'), f('be1'), f('m1'), f('v1'))
    W2, b2 = _fold_bn(f('w2'), f('b2'), f('g2'), f('be2'), f('m2'), f('v2'))
    W3, b3 = _fold_bn(f('w3'), f('b3'), f('g3'), f('be3'), f('m3'), f('v3'))
    sf = f('gf') / np.sqrt(f('vf') + EPS_BN)
    wf2 = (f('wf') * sf[:, None]).T              # [K=128, F=128]
    bf2 = (f('bf') - f('mf')) * sf + f('bef')
    c1 = b1
    c2 = b2 + W2.sum(axis=2) @ c1
    c3 = b3 + W3.sum(axis=2) @ c2
    bf3 = bf2 + wf2.T @ c3
    return dict(W1=W1, W2=W2, W3=W3, wf2=wf2, c1=c1, c2=c2, c3=c3, bf3=bf3)


def _layouts(p):
    w1 = np.zeros((128, 3 * 128), np.float64)
    for r in range(4):
        for t in range(3):
            for a in range(4):
                w1[32 * r + 8 * a: 32 * r + 8 * a + 8,
                   128 * t + 32 * a: 128 * t + 32 * a + 32] = p['W1'][:, :, t].T
    w2 = np.zeros((128, 3 * 64), np.float64)
    for s in range(4):
        for t in range(3):
            w2[32 * s: 32 * s + 32, 64 * t: 64 * t + 64] = p['W2'][:, :, t].T
    w3 = np.zeros((128, 3 * 128), np.float64)
    for h in range(2):
        for t in range(3):
            w3[64 * h: 64 * h + 64, 128 * t: 128 * t + 128] = p['W3'][:, :, t].T
    out = dict(w1=w1, w2=w2, w3=w3, wf=p['wf2'] / 64.0)
    out['negb1'] = np.tile(-p['c1'], 4)[:, None]
    out['negc2'] = np.tile(-p['c2'], 2)[:, None]
    out['negc3'] = (-p['c3'])[:, None]
    out['negc2_5'] = np.tile(out['negc2'], (1, 5))
    out['posb1'] = -out['negb1']
    out['posc2'] = -out['negc2']
    out['posc3'] = -out['negc3']
    out['negb1_4'] = np.tile(out['negb1'], (1, 4))
    out['bf3_row'] = p['bf3'][None, :]
    out['bf3_col'] = p['bf3'][:, None]
    return out


def _build_onehot_core(s_lab_core):
    """oh dram layout [128, B_LOC*2*20], scaled by -2 so the prototype
    matmul (lhsT=sf, rhs=inv*oh) directly yields pm2 = -2*protos^T.

    Row r of block (e, c) corresponds to the fc-lhsT stream order
    (a, h, b): support index s = 4*(a + 32*c) + 2*b + h."""
    oh = np.zeros((128, B_LOC * 2 * N_WAY), np.float64)
    for e in range(B_LOC):
        lab = np.asarray(s_lab_core[e])
        onehot = (lab[:, None] == np.arange(N_WAY)[None, :]).astype(np.float64)
        counts = onehot.sum(axis=0)
        counts[counts == 0] = 1.0
        ohn = -2.0 * onehot / counts[None, :]    # [200, 20]
        for c, (a0, na) in enumerate([(0, 32), (32, 18)]):
            blk = np.zeros((128, N_WAY))
            for r in range(4 * na):
                a, hb = a0 + r // 4, r % 4
                h, b = hb // 2, hb % 2
                blk[r] = ohn[4 * a + 2 * b + h]
            oh[:, (2 * e + c) * N_WAY:(2 * e + c + 1) * N_WAY] = blk
    return oh


# ---------------- device graph ----------------

_CACHE = {}


def _build_nc():
    import concourse.bass as bass
    import concourse.tile as tile
    from concourse import bacc, mybir
    from contextlib import ExitStack
    dt = mybir.dt
    AO = mybir.AluOpType
    AF = mybir.ActivationFunctionType

    nc = bacc.Bacc("TRN2", target_bir_lowering=False, debug=False,
                   num_devices=NCORES)

    dram = {}
    def din(name, shape, dtype):
        dram[name] = nc.dram_tensor(name, list(shape), dtype, kind="ExternalInput")

    din('imgs', (NSLOT, C_IN, L0 + 2), dt.float16)
    din('w1', (128, 384), dt.float16)
    din('w2', (128, 192), dt.float16)
    din('w3', (128, 384), dt.float16)
    din('wf', (128, 128), dt.float32)
    din('negb1', (128, 1), dt.float32)
    din('posb1', (128, 1), dt.float32)
    din('posc2', (128, 1), dt.float32)
    din('posc3', (128, 1), dt.float32)
    din('negb1_4', (128, 4), dt.float16)
    din('negc2', (128, 1), dt.float32)
    din('negc3', (128, 1), dt.float32)
    din('negc2_5', (128, 5), dt.float16)
    din('bf3_row', (1, 128), dt.float32)
    din('bf3_col', (128, 1), dt.float32)
    din('oh', (128, B_LOC * 2 * N_WAY), dt.float32)
    din('onesrow_f', (1, 128), dt.float32)
    din('ones128_f', (128, 1), dt.float32)
    u_d = nc.dram_tensor('u', [NQ, B_LOC * N_WAY], dt.float32,
                         kind="ExternalOutput")
    nsq_d = nc.dram_tensor('nsq', [1, N_QRY], dt.float32,
                           kind="ExternalOutput")
    pm2_d = nc.dram_tensor('pm2', [128, B_LOC * N_WAY], dt.float32,
                           kind="ExternalOutput")

    with tile.TileContext(nc) as tc, ExitStack() as ctx:
        cpool = ctx.enter_context(tc.tile_pool(name="consts", bufs=1))
        cs = {}
        # DMA order = first-use order so the pipeline starts ASAP
        for name, shape, d in [
            ('w1', (128, 384), dt.float16), ('negb1_4', (128, 4), dt.float16),
            ('posb1', (128, 1), dt.float32), ('negb1', (128, 1), dt.float32),
            ('w2', (128, 192), dt.float16), ('negc2_5', (128, 5), dt.float16),
            ('posc2', (128, 1), dt.float32), ('negc2', (128, 1), dt.float32),
            ('w3', (128, 384), dt.float16),
            ('posc3', (128, 1), dt.float32), ('negc3', (128, 1), dt.float32),
            ('wf', (128, 128), dt.float32),
            ('bf3_row', (1, 128), dt.float32), ('bf3_col', (128, 1), dt.float32),
            ('oh', (128, B_LOC * 2 * N_WAY), dt.float32),
            ('onesrow_f', (1, 128), dt.float32), ('ones128_f', (128, 1), dt.float32),
        ]:
            t = cpool.tile(list(shape), d, tag=f"c_{name}")
            nc.sync.dma_start(out=t[:], in_=dram[name].ap())
            cs[name] = t
        # dummy sqrt at t=0: front-load the ACT table set that holds both
        # Sqrt (protos) and the Relu/Square fillers, avoiding a ~2.7us
        # mid-kernel table switch
        warm = cpool.tile([1, 1], dt.float32, tag="warm")
        nc.scalar.activation(out=warm[0:1, 0:1], in_=cs['onesrow_f'][0:1, 0:1],
                             func=AF.Sqrt)

        means = cpool.tile([128, NSLOT], dt.float32, tag="means")
        # means column view by (a, ti, h, s, b): col = 16a+8ti+4h+2s+b
        mview = means[:].rearrange("p (a ti h s b) -> p a ti h s b",
                                   ti=2, h=2, s=2, b=2)

        img_pool = ctx.enter_context(tc.tile_pool(name="img", bufs=4))
        p1_pool = ctx.enter_context(tc.tile_pool(name="p1", bufs=2))
        l3r_pool = ctx.enter_context(tc.tile_pool(name="l3r", bufs=2))
        p3_pool = ctx.enter_context(tc.tile_pool(name="p3", bufs=3))
        str_pool = ctx.enter_context(tc.tile_pool(name="str", bufs=3))
        tmp_pool = ctx.enter_context(tc.tile_pool(name="tmp", bufs=8))
        ps_pool = ctx.enter_context(tc.tile_pool(name="ps", bufs=4, space="PSUM"))
        ep_pool = ctx.enter_context(tc.tile_pool(name="ep", bufs=2))
        sf_pool = ctx.enter_context(tc.tile_pool(name="sf", bufs=2))
        pm2_pool = ctx.enter_context(tc.tile_pool(name="pm2", bufs=4))
        q_pool = ctx.enter_context(tc.tile_pool(name="q", bufs=1))

        imgs, p1s, Ts, p3s = {}, {}, {}, {}

        def emit_load(b):
            img = img_pool.tile([128, L0 + 2], dt.float16, tag="img")
            nc.sync.dma_start(out=img[:, :],
                              in_=dram['imgs'].ap()[b * 16:b * 16 + 16]
                              .rearrange("i c l -> (i c) l"))
            imgs[b] = img

        def emit_conv1(b):
            img = imgs.pop(b)
            p1 = p1_pool.tile([128, 4 * 258], dt.float16, tag="pooled1")
            p1v = p1[:].rearrange("p (r c) -> p r c", r=4)
            if b < 2:
                # guard columns: written once per slot, persist across reuse
                nc.gpsimd.tensor_copy(out=p1v[:, :, 0], in_=cs['negb1_4'][:, 0:4])
                nc.gpsimd.tensor_copy(out=p1v[:, :, 257], in_=cs['negb1_4'][:, 0:4])
            for rp in range(2):
                ps1 = ps_pool.tile([128, 1024], dt.float32, tag="ps")
                for j in range(2):
                    r = 2 * rp + j
                    for d in range(3):
                        nc.tensor.matmul(
                            out=ps1[:, 512 * j:512 * j + 512],
                            lhsT=cs['w1'][32 * r:32 * r + 32,
                                          128 * d:128 * d + 128],
                            rhs=img[32 * r:32 * r + 32, d:d + 512],
                            start=(d == 0), stop=(d == 2),
                            tile_position=(32 * r, 0))
                v1 = ps1.rearrange("p (j l) -> p j l", j=2)
                t1 = tmp_pool.tile([128, 512], dt.float16, tag="tmp")
                t1v = t1[:].rearrange("p (j l) -> p j l", j=2)
                nc.scalar.activation(out=t1v[:, :, :], in_=v1[:, :, 0:512:2],
                                     func=AF.Relu, bias=cs['posb1'][:, 0:1])
                nc.vector.scalar_tensor_tensor(
                    out=p1v[:, 2 * rp:2 * rp + 2, 1:257], in0=t1v[:, :, :],
                    scalar=cs['negb1'][:, 0:1], in1=v1[:, :, 1:512:2],
                    op0=AO.add, op1=AO.max)
            p1s[b] = p1

        def emit_conv2(b):
            p1 = p1s.pop(b)
            p1v = p1[:].rearrange("p (r c) -> p r c", r=4)
            T = l3r_pool.tile([128, 1036], dt.float16, tag="l3rhs")
            if b < 2:
                for ti in range(2):
                    nc.gpsimd.tensor_copy(out=T[:, 518 * ti:518 * ti + 517:129],
                                          in_=cs['negc2_5'][:, 0:5])
            for t2i in range(2):
                ps2 = ps_pool.tile([128, 1024], dt.float32, tag="ps")
                for sj in range(2):
                    s = 2 * t2i + sj
                    p1sv = p1v[32 * s:32 * s + 32]
                    for c in range(2):
                        for d in range(3):
                            nc.tensor.matmul(
                                out=ps2[64 * c:64 * c + 64,
                                        512 * sj:512 * sj + 512],
                                lhsT=cs['w2'][32 * s:32 * s + 32,
                                              64 * d:64 * d + 64],
                                rhs=p1sv[:, c:4:2, d:d + 256],
                                start=(d == 0), stop=(d == 2),
                                tile_position=(32 * s, 64 * c))
                src2 = ps2.rearrange("p (x b l) -> p x b l", x=2, b=2)
                t2 = tmp_pool.tile([128, 512], dt.float16, tag="tmp")
                t2v = t2[:].rearrange("p (x b l) -> p x b l", x=2, b=2)
                nc.scalar.activation(out=t2v[:, :, :, :],
                                     in_=src2[:, :, :, 0:256:2],
                                     func=AF.Relu, bias=cs['posc2'][:, 0:1])
                dst = T[:, 518 * t2i + 1:518 * t2i + 517].rearrange(
                    "p (x b l) -> p x b l", x=2, b=2)
                nc.vector.scalar_tensor_tensor(
                    out=dst[:, :, :, 0:128], in0=t2v[:, :, :, :],
                    scalar=cs['negc2'][:, 0:1], in1=src2[:, :, :, 1:256:2],
                    op0=AO.add, op1=AO.max)
            Ts[b] = T

        def emit_conv3(b):
            T = Ts.pop(b)
            p3 = p3_pool.tile([128, 1024], dt.float32, tag="p3")
            for ti in range(2):
                ps3 = ps_pool.tile([128, 1024], dt.float32, tag="ps")
                for h in range(2):
                    rhs3 = T[64 * h:64 * h + 64, 518 * ti:518 * ti + 518]
                    for d in range(3):
                        rv = rhs3[:, d:d + 516].rearrange(
                            "p (k l) -> p k l", k=4)[:, :, 0:128]
                        nc.tensor.matmul(
                            out=ps3[:, 512 * h:512 * h + 512],
                            lhsT=cs['w3'][64 * h:64 * h + 64,
                                          128 * d:128 * d + 128],
                            rhs=rv,
                            start=(d == 0), stop=(d == 2),
                            tile_position=(64 * h, 0))
                s3v = ps3.rearrange("p (h k l) -> p h k l", h=2, k=4)
                t3 = tmp_pool.tile([128, 512], dt.float16, tag="tmp")
                t3v = t3[:].rearrange("p (h k l) -> p h k l", h=2, k=4)
                nc.scalar.activation(out=t3v[:, :, :, :],
                                     in_=s3v[:, :, :, 0:128:2],
                                     func=AF.Relu, bias=cs['posc3'][:, 0:1])
                d3 = p3[:, 512 * ti:512 * ti + 512].rearrange(
                    "p (h k l) -> p h k l", h=2, k=4)
                nc.vector.scalar_tensor_tensor(
                    out=d3[:, :, :, 0:64], in0=t3v[:, :, :, :],
                    scalar=cs['negc3'][:, 0:1], in1=s3v[:, :, :, 1:128:2],
                    op0=AO.add, op1=AO.max)
            p3s[b] = p3

        def emit_tree(bb):
            """avgpool add-tree for burst bb (p3 -> means cols m-order)."""
            p3 = p3s.pop(bb)
            p3v = p3.rearrange("p (m l) -> p m l", m=16)
            s = str_pool.tile([128, 896], dt.float32, tag="scratch")
            s1 = s[:, 0:512].rearrange("p (m l) -> p m l", m=16)
            s2 = s[:, 512:768].rearrange("p (m l) -> p m l", m=16)
            s3 = s[:, 768:896].rearrange("p (m l) -> p m l", m=16)
            nc.gpsimd.tensor_tensor(out=s1[:, :, :], in0=p3v[:, :, 0:32],
                                    in1=p3v[:, :, 32:64], op=AO.add)
            nc.gpsimd.tensor_tensor(out=s2[:, :, :], in0=s1[:, :, 0:16],
                                    in1=s1[:, :, 16:32], op=AO.add)
            nc.gpsimd.tensor_tensor(out=s3[:, :, :], in0=s2[:, :, 0:8],
                                    in1=s2[:, :, 8:16], op=AO.add)
            nc.vector.tensor_reduce(out=means[:, 16 * bb:16 * bb + 16],
                                    in_=s3[:, :, :],
                                    axis=mybir.AxisListType.X, op=AO.add)

        # ---------------- epilogue: protos in 4 spread sub-stages ----------

        pr = {}

        def emit_proto_stage(st):
            if st == 0:
                # stage all 4 episodes' support means contiguously:
                # col (a,h,b) -> support index 4a+2b+h
                stage = ep_pool.tile([128, 4 * 200], dt.float32, tag="stage")
                for e in range(B_LOC):
                    tie, se = (e >> 1) & 1, e & 1
                    nc.gpsimd.tensor_copy(
                        out=stage[:, 200 * e:200 * e + 200].rearrange(
                            "p (a h b) -> p a h b", h=2, b=2),
                        in_=mview[:, 0:50, tie, :, se, :])
                pr['stage'] = stage
            elif st == 1:
                # 8 fc matmuls (block = 2e+c) + bias via K=1 accumulating
                # matmul, all into one 2-bank hold; one relu drains it
                hold = ps_pool.tile([128, 1024], dt.float32, tag="ps")
                for e in range(B_LOC):
                    for c, (off, sz) in enumerate([(0, 128), (128, 72)]):
                        blk = 2 * e + c
                        nc.tensor.matmul(
                            out=hold[0:sz, 128 * blk:128 * blk + 128],
                            lhsT=pr['stage'][:, 200 * e + off:200 * e + off + sz],
                            rhs=cs['wf'][:, 0:128], start=True, stop=False)
                        nc.tensor.matmul(
                            out=hold[0:sz, 128 * blk:128 * blk + 128],
                            lhsT=cs['onesrow_f'][0:1, 0:sz],
                            rhs=cs['bf3_row'][0:1, 0:128],
                            start=False, stop=True)
                sf = sf_pool.tile([128, 1024], dt.float32, tag="sf")
                nc.scalar.activation(out=sf[:, :], in_=hold[0:128, 0:1024],
                                     func=AF.Relu)
                pr['sf'] = sf
            elif st == 2:
                sq = ep_pool.tile([128, 1024], dt.float32, tag="sq")
                nc.gpsimd.tensor_tensor(out=sq[:, :], in0=pr['sf'][:, :],
                                        in1=pr['sf'][:, :], op=AO.mult)
                nsq = ep_pool.tile([128, 8], dt.float32, tag="nsq")
                nc.vector.tensor_reduce(
                    out=nsq[:, :], in_=sq[:].rearrange("p (k f) -> p k f", k=8),
                    axis=mybir.AxisListType.X, op=AO.add)
                nrm = ep_pool.tile([128, 8], dt.float32, tag="nrm")
                nc.scalar.activation(out=nrm[:, :], in_=nsq[:, :], func=AF.Sqrt)
                inv = ep_pool.tile([128, 8], dt.float32, tag="inv")
                nc.vector.reciprocal(out=inv[:, :], in_=nrm[:, :])
                pr['inv'] = inv
            elif st == 3:
                # scale the one-hot by the per-support inv-norm instead of
                # normalizing the features: pm2 = sf^T @ (inv * oh)
                ohs = ep_pool.tile([128, 8 * N_WAY], dt.float32, tag="ohs")
                for blk in range(8):
                    sz = 128 if blk % 2 == 0 else 72
                    nc.vector.tensor_scalar(
                        out=ohs[0:sz, N_WAY * blk:N_WAY * blk + N_WAY],
                        in0=cs['oh'][0:sz, N_WAY * blk:N_WAY * blk + N_WAY],
                        scalar1=pr['inv'][0:sz, blk:blk + 1], scalar2=None,
                        op0=AO.mult)
                pr['ohs'] = ohs
            elif st == 4:
                hold = ps_pool.tile([128, 1024], dt.float32, tag="ps")
                for e in range(B_LOC):
                    for c, sz in [(0, 128), (1, 72)]:
                        blk = 2 * e + c
                        nc.tensor.matmul(
                            out=hold[0:128, N_WAY * e:N_WAY * e + N_WAY],
                            lhsT=pr['sf'][0:sz, 128 * blk:128 * blk + 128],
                            rhs=pr['ohs'][0:sz,
                                          N_WAY * blk:N_WAY * blk + N_WAY],
                            start=(c == 0), stop=(c == 1))
                pm2 = pm2_pool.tile([128, B_LOC * N_WAY], dt.float32, tag="pm2")
                nc.vector.tensor_copy(out=pm2[:, :],
                                      in_=hold[0:128, 0:B_LOC * N_WAY])
                nc.sync.dma_start(out=pm2_d.ap(), in_=pm2[:, :])
                pr['pm2'] = pm2

        def emit_query_tail():
            """Batched query tail: one fc/relu/sq/norm for all 4 episodes,
            then 4 tiny dp matmuls. Final elementwise math is host-side."""
            qstage = q_pool.tile([128, N_QRY], dt.float32, tag="qstage")
            for ti in range(2):
                ov = qstage[:, 200 * ti:200 * ti + 200].rearrange(
                    "p (s a h b) -> p a h s b", a=25, h=2, b=2)
                nc.vector.tensor_copy(out=ov, in_=mview[:, 50:75, ti, :, :, :])
            hold = ps_pool.tile([128, 1024], dt.float32, tag="ps")
            nc.tensor.matmul(out=hold[0:128, 0:N_QRY],
                             lhsT=cs['wf'][:, 0:128],
                             rhs=qstage[:, 0:N_QRY], start=True, stop=True)
            qf = q_pool.tile([128, N_QRY], dt.float32, tag="qf")
            nc.scalar.activation(out=qf[:, :], in_=hold[0:128, 0:N_QRY],
                                 func=AF.Relu, bias=cs['bf3_col'][:, 0:1])
            qsq = q_pool.tile([128, N_QRY], dt.float32, tag="qsq")
            nc.vector.tensor_tensor(out=qsq[:, :], in0=qf[:, :],
                                    in1=qf[:, :], op=AO.mult)
            hold2 = ps_pool.tile([128, 1024], dt.float32, tag="ps")
            nc.tensor.matmul(out=hold2[0:1, 0:N_QRY],
                             lhsT=cs['ones128_f'][:, 0:1],
                             rhs=qsq[:, :], start=True, stop=True)
            nsq_sb = q_pool.tile([1, N_QRY], dt.float32, tag="nsq_sb")
            nc.vector.tensor_copy(out=nsq_sb[:, :], in_=hold2[0:1, 0:N_QRY])
            nc.sync.dma_start(out=nsq_d.ap(), in_=nsq_sb[:, :])
            hold3 = ps_pool.tile([128, 1024], dt.float32, tag="ps")
            for e in range(B_LOC):
                nc.tensor.matmul(out=hold3[0:NQ, N_WAY * e:N_WAY * e + N_WAY],
                                 lhsT=qf[:, NQ * e:NQ * e + NQ],
                                 rhs=pr['pm2'][:, N_WAY * e:N_WAY * e + N_WAY],
                                 start=True, stop=True)
            u_sb = q_pool.tile([128, B_LOC * N_WAY], dt.float32, tag="u_sb")
            nc.vector.tensor_copy(out=u_sb[0:NQ, :],
                                  in_=hold3[0:NQ, 0:B_LOC * N_WAY])
            nc.sync.dma_start(out=u_d.ap(), in_=u_sb[0:NQ, :])

        # ---------------- software-pipelined main loop ----------------
        proto_at = {52: [0], 53: [1], 54: [2], 55: [3], 56: [4]}

        emit_load(0)
        emit_load(1)
        for s in range(NBURST + 3):
            if s + 2 < NBURST:
                emit_load(s + 2)
            if s < NBURST:
                emit_conv1(s)
            if 0 <= s - 1 < NBURST:
                emit_conv2(s - 1)
            if 0 <= s - 2 < NBURST:
                emit_conv3(s - 2)
            if 0 <= s - 3 < NBURST:
                emit_tree(s - 3)
            for st in proto_at.get(s, []):
                emit_proto_stage(st)
        emit_query_tail()

    nc.compile()
    return nc


def _host_inputs(inputs):
    p = _prepare(inputs)
    lay = _layouts(p)
    f32 = lambda a: np.ascontiguousarray(a, dtype=np.float32)
    b16 = lambda a: np.ascontiguousarray(np.asarray(a, np.float32).astype(F16))
    s_img = np.asarray(inputs['s_img'], np.float32)
    q_img = np.asarray(inputs['q_img'], np.float32)
    s_lab = np.asarray(inputs['s_lab'])
    common = {
        'w1': b16(lay['w1']), 'w2': b16(lay['w2']), 'w3': b16(lay['w3']),
        'wf': f32(lay['wf']),
        'negb1': f32(lay['negb1']), 'negc2': f32(lay['negc2']),
        'posb1': f32(lay['posb1']), 'posc2': f32(lay['posc2']),
        'posc3': f32(lay['posc3']), 'negb1_4': b16(lay['negb1_4']),
        'negc3': f32(lay['negc3']), 'negc2_5': b16(lay['negc2_5']),
        'bf3_row': f32(lay['bf3_row']), 'bf3_col': f32(lay['bf3_col']),
        'onesrow_f': f32(np.ones((1, 128))), 'ones128_f': f32(np.ones((128, 1))),
    }
    in_maps = []
    for i in range(NCORES):
        e0 = i * B_LOC
        m = dict(common)
        # slot 4*i+e <- image i of episode e (episode-interleaved)
        sup = s_img[e0:e0 + B_LOC].transpose(1, 0, 2, 3).reshape(N_SUP, C_IN, L0)
        qry = q_img[e0:e0 + B_LOC].transpose(1, 0, 2, 3).reshape(N_QRY, C_IN, L0)
        allimg = np.concatenate([sup, qry], axis=0)
        padded = np.zeros((NSLOT, C_IN, L0 + 2), np.float32)
        padded[:, :, 1:L0 + 1] = allimg
        m['imgs'] = b16(padded)
        m['oh'] = f32(_build_onehot_core(s_lab[e0:e0 + B_LOC]))
        in_maps.append(m)
    return in_maps


# query col c = 4a+2h+b for query q = 4a+2b+h (swap h<->b in the low bits)
_QCOL = np.array([4 * (q // 4) + 2 * (q & 1) + ((q >> 1) & 1)
                  for q in range(NQ)], dtype=np.int64)


def _host_finish(res_core):
    """Assemble -dist [B_LOC, NQ, N_WAY] from raw device outputs."""
    u = np.asarray(res_core['u'], np.float64)        # [NQ, 4*20], rows=col c
    nsq = np.asarray(res_core['nsq'], np.float64)[0]  # [400], (e-block, col c)
    pm2 = np.asarray(res_core['pm2'], np.float64)     # [128, 4*20]
    out = np.empty((B_LOC, NQ, N_WAY), np.float32)
    for e in range(B_LOC):
        rows = 0.25 * (pm2[:, 20 * e:20 * e + 20] ** 2).sum(axis=0) + 1.0
        inv = 1.0 / np.sqrt(np.maximum(nsq[100 * e + _QCOL], 1e-24))
        d2 = u[_QCOL, 20 * e:20 * e + 20] * inv[:, None] + rows[None, :]
        out[e] = -np.sqrt(np.maximum(d2, 0.0))
    return out


def _ensure_ntff_hook():
    try:
        from antenv.axon_hooks import (get_axon_ntff_profile_hook,
                                       set_axon_ntff_profile_hook)
        if get_axon_ntff_profile_hook() is None:
            from trn_agent_boot.trn_boot import _ntff_profile_via_ctypes
            set_axon_ntff_profile_hook(
                _ntff_profile_via_ctypes('/opt/axon/libaxon_pjrt.so'))
    except Exception as e:
        print('ntff hook setup failed:', e)


def _run(inputs, trace=False):
    from concourse.bass_utils import run_bass_kernel_spmd
    if trace:
        _ensure_ntff_hook()
    if 'nc' not in _CACHE:
        _CACHE['nc'] = _build_nc()
    nc = _CACHE['nc']
    in_maps = _host_inputs(inputs)
    res = run_bass_kernel_spmd(nc, in_maps, core_ids=list(range(NCORES)),
                               trace=trace)
    outs = [_host_finish(res.results[i]) for i in range(NCORES)]
    full = np.concatenate(outs, axis=0).astype(np.float32)
    return full, res


def kernel(**inputs):
    out, _ = _run(inputs, trace=False)
    return out


def run_traced(**inputs):
    return _run(inputs, trace=True)
